# revision 33
# baseline (speedup 1.0000x reference)
"""Trainium2 Bass kernel for nn_BDH_39127152067244 (dense_transformer).

Sharding: 8 cores = (b, h) pairs — b = core // 4, h = core % 4. Each core
computes its head's share of every layer; the only cross-core communication
is a 4-rank AllReduce of the per-head yMLP partial (replica groups {0..3}
and {4..7}), issued once per t-half per layer.

Structure: each layer is emitted as two serialized t-half streams
(t in [0,256) then [256,512)). The AllReduce for half h is issued right
after that half's decoder matmul finishes and is consumed only at the START
of the next layer's half-h stream — a full opposite-half stream (~60us) is
always in flight between issue and use, so collective latency is hidden and
the PE never idles long enough to drop to the cold HAM clock. A dummy
AllReduce at kernel start absorbs the expensive first-collective path.

Layout tricks (vs the reference):
  - The N axis (8192) is deinterleaved on the host (even n first, odd n
    second). Rope's interleaved pair-swap becomes a half-offset of whole
    128-partition tiles. Both rope tables are then column-periodic with
    period N/2, so only [T, 4096] of cos and sin are stored/streamed.
  - x_sparse lives in ONE [P, 8, 8, TH] tile in A_VG order so each rope
    pair's two V-blocks are adjacent; rope runs as two merged [P,2,4,TH]
    muls against the natural/block-reversed table view plus two combines.
  - x_sparse is computed directly in transposed [N, T] layout.
  - Per-jl PSUM tiles for yKV and the lm head (Tile's PE-write vs
    engine-read hazard is tile-granular; separate tiles let jl1 matmuls
    overlap jl0's LN). LN rstd is one fused Abs_reciprocal_sqrt ACTIVATE;
    LN applies run on ScalarE with a precomputed -mean*rstd bias.
  - encv/dec chunks prefetch at half start; enc loads in 8 chunks ordered
    as A consumes them, with HAM-warming filler transposes during startup.
  - scores: the Gram matrix of rope'd activations is symmetric, so the
    strict-lower-triangular masked scores in [t, s] layout equal the
    strict-upper masked Gram in [s, t] layout — computed directly as the
    yKV matmul's lhsT. Only diagonal 128x128 blocks are masked; fully-kept
    blocks are copied and fully-masked blocks never computed.
  - All matmuls run in bf16 with f32 PSUM accumulation; LayerNorms and the
    residual stream stay f32.
"""

import math
import sys
from contextlib import ExitStack

import numpy as np
import ml_dtypes

sys.path.insert(0, "/opt/trn_rl_repo")

import concourse.bass as bass  # noqa: E402
import concourse.bacc as bacc  # noqa: E402
import concourse.mybir as mybir  # noqa: E402
import concourse.tile as tile  # noqa: E402
from concourse.bass import ds  # noqa: E402
from concourse.bass_utils import run_bass_kernel_spmd  # noqa: E402
from concourse.masks import make_identity  # noqa: E402

BF16 = ml_dtypes.bfloat16
BF = mybir.dt.bfloat16
FP32 = mybir.dt.float32
AF = mybir.ActivationFunctionType
ALU = mybir.AluOpType

# Problem constants (hardcoded per the harness contract).
N_LAYER = 6
D = 256
NH = 4
N = 8192
HALF = N // 2
VOCAB = 256
B, T = 2, 512
THETA = 2.0**16
EPS = 1e-5

P = 128          # partitions
NT = N // P      # 64 n-tiles
G4 = 4           # n-tiles per rope group
NG = NT // G4    # 16 groups
VG = 8           # n-tiles per V tile
NVG = NT // VG   # 8 V tiles
TC = T // P      # 4 t-chunks
TH = T // 2      # 256 columns per half
DT = D // P      # 2 d-tiles
N_CORES = 8
RG = [[0, 1, 2, 3], [4, 5, 6, 7]]

# rope pair emission order: (g, g^8) adjacent so the shared table tile is
# used twice back to back; A-phase V-tile order matches.
PAIR_G = [0, 8, 1, 9, 2, 10, 3, 11, 4, 12, 5, 13, 6, 14, 7, 15]
A_VG = [0, 4, 1, 5, 2, 6, 3, 7]
A_IDX = [A_VG.index(v) for v in range(8)]  # vg value -> V_all slot
GP_OFF = 0  # h0 rope pair-groups whose sin-mul runs on gpsimd (2.6us/op
# on gpsimd vs 0.59 on DVE made it the pacer — keep 0)

_CACHE: dict = {}


def _build_bass():
    nc = bacc.Bacc("TRN2", num_devices=N_CORES)

    x0_d = nc.dram_tensor("x0", [P, TC, D], FP32, kind="ExternalInput")
    x0bf_d = nc.dram_tensor("x0bf", [P, TC, D], BF, kind="ExternalInput")
    x0T_d = nc.dram_tensor("x0T", [P, DT, T], BF, kind="ExternalInput")
    enc_d = nc.dram_tensor("enc", [P, DT, NT, P], BF, kind="ExternalInput")
    encv_d = nc.dram_tensor("encv", [P, DT, NT, P], BF, kind="ExternalInput")
    dec_d = nc.dram_tensor("dec", [P, NT, D], BF, kind="ExternalInput")
    tab_d = nc.dram_tensor("tabb", [P, 8, 2 * G4, T], BF, kind="ExternalInput")
    mask_d = nc.dram_tensor("maskb", [P, P], BF, kind="ExternalInput")
    lm_d = nc.dram_tensor("lm", [P, DT, VOCAB], BF, kind="ExternalInput")
    out_d = nc.dram_tensor("logits", [P, TC, VOCAB], FP32, kind="ExternalOutput")

    with tile.TileContext(nc) as tc, ExitStack() as ctx:
        sb = ctx.enter_context(tc.tile_pool(name="sb", bufs=1))
        wres = ctx.enter_context(tc.tile_pool(name="wres", bufs=1))
        vpool = ctx.enter_context(tc.tile_pool(name="vpool", bufs=8))
        qr0p = ctx.enter_context(tc.tile_pool(name="qr0p", bufs=8))
        qr1p = ctx.enter_context(tc.tile_pool(name="qr1p", bufs=2))
        tmpp = ctx.enter_context(tc.tile_pool(name="tmpp", bufs=2))
        tabp = ctx.enter_context(tc.tile_pool(name="tabp", bufs=4))
        evp = ctx.enter_context(tc.tile_pool(name="evp", bufs=2))
        decp = ctx.enter_context(tc.tile_pool(name="decp", bufs=2))
        stp = ctx.enter_context(tc.tile_pool(name="stp", bufs=1))
        yp = ctx.enter_context(tc.tile_pool(name="yp", bufs=2))
        xyp = ctx.enter_context(tc.tile_pool(name="xyp", bufs=2))
        xp = ctx.enter_context(tc.tile_pool(name="xp", bufs=2))
        statp = ctx.enter_context(tc.tile_pool(name="statp", bufs=4))
        aqp = ctx.enter_context(tc.tile_pool(name="aqp", bufs=2, space="PSUM"))
        mmp = ctx.enter_context(tc.tile_pool(name="mmp", bufs=2, space="PSUM"))
        drm = ctx.enter_context(tc.tile_pool(name="drm", bufs=2, space="DRAM"))

        ident = sb.tile([P, P], BF, name="ident")
        make_identity(nc, ident)
        epst = sb.tile([P, 1], FP32, name="epst")
        nc.vector.memset(epst, EPS)
        maskd = sb.tile([P, P], BF, name="maskd")
        nc.sync.dma_start(out=maskd, in_=mask_d[:])
        lmt = sb.tile([P, DT, VOCAB], BF, name="lmt")
        nc.sync.dma_start(out=lmt, in_=lm_d[:])

        enc_sb = wres.tile([P, DT, NT, P], BF, name="enc_sb")

        # state carried between phases: per half h -> tile
        ST = {"ymr": {}, "xf": {}, "xbf": {}, "xT": {}}

        def rstd_of(var_ap, name):
            r = statp.tile([P, 1], FP32, tag="rs", name=f"rs_{name}")
            # fused 1/sqrt(|var+eps|): one scalar op, no DVE hop (Rsqrt is
            # gated in bass; the abs variant isn't, and var+eps > 0 anyway)
            nc.scalar.activation(
                out=r, in_=var_ap, func=AF.Abs_reciprocal_sqrt, bias=epst
            )
            return r

        def neg_mb(mean_ap, r_ap, name):
            """-mean*rstd, the ACTIVATE bias for a fused LN apply."""
            nb = statp.tile([P, 1], FP32, tag="nb", name=f"nb_{name}")
            nc.vector.scalar_tensor_tensor(
                out=nb, in0=mean_ap, scalar=-1.0, in1=r_ap,
                op0=ALU.mult, op1=ALU.mult,
            )
            return nb

        def ln_stats(src_ap, name):
            s6 = statp.tile([P, 6], FP32, tag="bst", name=f"st_{name}")
            nc.vector.bn_stats(out=s6, in_=src_ap)
            mv = statp.tile([P, 2], FP32, tag="bmv", name=f"mv_{name}")
            nc.vector.bn_aggr(out=mv, in_=s6)
            return mv

        def emit_x_from_dram(h):
            xf = xp.tile([P, 2, D], FP32, tag=f"xf{h}", name=f"xf0_{h}")
            nc.sync.dma_start(out=xf, in_=x0_d[:, ds(h * 2, 2), :])
            xbf = xp.tile([P, 2, D], BF, tag=f"xbf{h}", name=f"xbf0_{h}")
            nc.sync.dma_start(out=xbf, in_=x0bf_d[:, ds(h * 2, 2), :])
            xT = xp.tile([P, DT, TH], BF, tag=f"xT{h}", name=f"xT0_{h}")
            nc.sync.dma_start(out=xT, in_=x0T_d[:, :, ds(h * TH, TH)])
            ST["xf"][h], ST["xbf"][h], ST["xT"][h] = xf, xbf, xT

        def emit_x_update(l, h):
            """x_new = LN(x_old + LN(ymr)); consumes AR(l-1, h)."""
            ymr = ST["ymr"][h]
            xf_old = ST["xf"][h]
            xf = xp.tile([P, 2, D], FP32, tag=f"xf{h}", name=f"xf{l}_{h}")
            xbf = xp.tile([P, 2, D], BF, tag=f"xbf{h}", name=f"xbf{l}_{h}")
            xmid = xp.tile([P, 2, D], FP32, tag="xmid", name=f"xm{l}_{h}", bufs=1)
            xT = xp.tile([P, DT, TH], BF, tag=f"xT{h}", name=f"xT{l}_{h}")
            tp4x = mmp.tile([P, 2, DT, P], BF, tag="mm", name=f"xtp{l}_{h}")
            for jl in range(2):
                mv1 = ln_stats(ymr[:, jl, :], f"y{l}_{h}_{jl}")
                r1 = rstd_of(mv1[:, 1:2], f"y{l}_{h}_{jl}")
                # mean-shift of LN(ymr) is absorbed by the outer LN
                nc.vector.scalar_tensor_tensor(
                    out=xmid[:, jl, :],
                    in0=ymr[:, jl, :],
                    scalar=r1,
                    in1=xf_old[:, jl, :],
                    op0=ALU.mult,
                    op1=ALU.add,
                )
                mv2 = ln_stats(xmid[:, jl, :], f"x{l}_{h}_{jl}")
                r2 = rstd_of(mv2[:, 1:2], f"x{l}_{h}_{jl}")
                nb2 = neg_mb(mv2[:, 0:1], r2, f"x{l}_{h}_{jl}")
                # LN apply as two scalar ACTIVATEs (fp32 + bf16 outputs):
                # frees the DVE and lets xbf (the transpose input) materialize
                # without waiting for xf
                nc.scalar.activation(
                    out=xbf[:, jl, :], in_=xmid[:, jl, :], func=AF.Identity,
                    bias=nb2, scale=r2,
                )
                nc.vector.tensor_scalar(
                    out=xf[:, jl, :],
                    in0=xmid[:, jl, :],
                    scalar1=mv2[:, 0:1],
                    scalar2=r2,
                    op0=ALU.subtract,
                    op1=ALU.mult,
                )
                # per-jl transposes: jl0's run on PE while jl1's LN chain is
                # still on the vector engine
                for dt_ in range(DT):
                    nc.tensor.transpose(
                        tp4x[:, jl, dt_, :], xbf[:, jl, ds(dt_ * P, P)], ident
                    )
            for dt_ in range(DT):
                nc.scalar.copy(
                    out=xT[:, dt_, :].rearrange("p (a b) -> p a b", a=2),
                    in_=tp4x[:, :, dt_, :],
                )
            ST["xf"][h], ST["xbf"][h], ST["xT"][h] = xf, xbf, xT

        def load_ev_chunk(l, h, c):
            """Stream one 16-n-tile chunk of encv+dec for half (l, h)."""
            evq = evp.tile([P, DT, 16, P], BF, tag="ev", name=f"ev{l}_{h}_{c}")
            nc.scalar.dma_start(out=evq, in_=encv_d[:, :, ds(c * 16, 16), :])
            decq = decp.tile([P, 16, D], BF, tag="dec", name=f"dq{l}_{h}_{c}")
            nc.scalar.dma_start(out=decq, in_=dec_d[:, ds(c * 16, 16), :])
            return evq, decq

        def emit_half(l, h):
            xT = ST["xT"][h]

            # prefetch this half's first two encv/dec chunks (their pool
            # slots were freed by the previous half's last chunks), so the
            # D -> E handoff never waits on HBM. For (0,0) defer until after
            # A's emission: at startup the HBM port is saturated loading enc
            # and the rope tables, which are needed first.
            EVC = {}
            if not (l == 0 and h == 0):
                EVC[0] = load_ev_chunk(l, h, 0)
                EVC[1] = load_ev_chunk(l, h, 1)

            # ---------- A: V = relu(enc^T @ x^T) on this half's columns ----
            # one tile, A_VG-ordered along dim 1 so each rope pair's two
            # V-blocks are adjacent and a single [P,2,4,TH] AP spans them
            vall = vpool.tile(
                [P, 8, VG, TH], BF, tag="v", name=f"v{l}_{h}", bufs=1
            )
            for vgi, vg in enumerate(A_VG):
                if l == 0 and h == 0 and vgi >= 1:
                    # HAM-warming filler while enc chunks stream in
                    for w in range(6):
                        wt0 = aqp.tile(
                            [P, 4, TH], BF, tag="aq", name=f"aw{vgi}_{w}"
                        )
                        nc.tensor.transpose(wt0[:, 0, 0:P], ident, ident)
                for q in range(2):
                    ps = aqp.tile([P, 4, TH], FP32, tag="aq", name=f"aps{l}_{h}_{vg}_{q}")
                    for j in range(4):
                        # enc storage order is A_VG order (host-reordered)
                        nt_ = vgi * VG + q * 4 + j
                        for dt_ in range(DT):
                            nc.tensor.matmul(
                                ps[:, j, :],
                                lhsT=enc_sb[:, dt_, nt_, :],
                                rhs=xT[:, dt_, :],
                                start=(dt_ == 0),
                                stop=(dt_ == DT - 1),
                            )
                    nc.scalar.activation(
                        out=vall[:, vgi, ds(q * 4, 4), :], in_=ps, func=AF.Relu
                    )
            if l == 0 and h == 0:
                EVC[0] = load_ev_chunk(l, h, 0)
                EVC[1] = load_ev_chunk(l, h, 1)

            # ---------- rope + Gram (C), merged per pair -------------------
            # pair j = rope groups (j, j+8); both share the (column-periodic)
            # table gm=j. The rotation runs as two merged [P,2,4,TH] muls
            # against the natural and block-reversed table views, then two
            # [P,4,TH] combines:
            #   m_a = [Vg|Vp]*[c|s],  m_b = [Vg|Vp]*[s|c]
            #   qr_g = m_a0 - m_a1,   qr_p = m_b0 + m_b1
            if h == 0:
                gps = mmp.tile([P, 2, TH], FP32, tag="mm", name=f"gps{l}_0")
            else:
                gps = mmp.tile([P, 4, TH], FP32, tag="mm", name=f"gps{l}_1")
            QR = {}
            for pj in range(8):
                tabg = tabp.tile(
                    [P, 2 * G4, TH], BF, tag="tab", name=f"tb{l}_{h}_{pj}"
                )
                nc.sync.dma_start(
                    out=tabg, in_=tab_d[:, pj, :, ds(h * TH, TH)]
                )
                tab2 = tabg[:].rearrange("p (a b) t -> p a b t", a=2)
                vpair = vall[:, ds(2 * (pj // 2), 2), ds((pj % 2) * G4, G4), :]
                pool = qr0p if h == 0 else qr1p
                qrp = pool.tile(
                    [P, 2, G4, TH], BF, tag=f"q{h}", name=f"qr{l}_{h}_{pj}"
                )
                QR[pj] = qrp
                c2 = tmpp.tile([P, 2, G4, TH], BF, tag="c2", name=f"c2_{l}_{h}_{pj}", bufs=1)
                s2 = tmpp.tile([P, 2, G4, TH], BF, tag="s2", name=f"s2_{l}_{h}_{pj}", bufs=1)
                nc.vector.tensor_mul(c2, vpair, tab2)
                nc.vector.tensor_mul(s2, vpair, tab2[:, ::-1, :, :])
                nc.vector.tensor_sub(qrp[:, 0], c2[:, 0], c2[:, 1])
                nc.vector.tensor_add(qrp[:, 1], s2[:, 0], s2[:, 1])

                # Gram accumulation for this pair's 8 k-tiles
                # NOTE: start=True clears has_written bits for the WHOLE psum
                # bank, so only the first matmul touching each bank may set it;
                # sibling regions in the same bank start with start=False and
                # rely on that clear (their first write then overwrites).
                # gps h0 [P,2,TH] = 1 bank; gps h1 [P,4,TH] = 2 banks (j01/j23).
                for hop in range(2):
                    for i in range(G4):
                        first = pj == 0 and hop == 0 and i == 0
                        stp_ = pj == 7 and hop == 1 and i == G4 - 1
                        qr = qrp[:, hop]
                        if h == 0:
                            # rows j=0 (cols 0:256), j=1 (cols 128:256)
                            nc.tensor.matmul(
                                gps[:, 0, :],
                                lhsT=qr[:, i, 0:P],
                                rhs=qr[:, i, :],
                                start=first,
                                stop=stp_,
                            )
                            nc.tensor.matmul(
                                gps[:, 1, P:TH],
                                lhsT=qr[:, i, P:TH],
                                rhs=qr[:, i, P:TH],
                                start=False,
                                stop=stp_,
                            )
                        else:
                            qr0 = _QR0[pj][:, hop]
                            for j in range(4):
                                lhs = (
                                    qr0[:, i, ds((j % 2) * P, P)]
                                    if j < 2
                                    else qr[:, i, ds((j - 2) * P, P)]
                                )
                                st_ = first and (j % 2 == 0)
                                if j < 3:
                                    nc.tensor.matmul(
                                        gps[:, j, :],
                                        lhsT=lhs,
                                        rhs=qr[:, i, :],
                                        start=st_,
                                        stop=stp_,
                                    )
                                else:
                                    nc.tensor.matmul(
                                        gps[:, 3, P:TH],
                                        lhsT=lhs,
                                        rhs=qr[:, i, P:TH],
                                        start=st_,
                                        stop=stp_,
                                    )
            if h == 0:
                _QR0.clear()
                _QR0.update(QR)

            # ---------- masked score extraction ----------------------------
            if h == 0:
                st0 = stp.tile([P, 2, TH], BF, tag="st0", name=f"st0_{l}")
                nc.vector.tensor_mul(st0[:, 0, 0:P], gps[:, 0, 0:P], maskd)
                nc.vector.tensor_mul(st0[:, 1, P:TH], gps[:, 1, P:TH], maskd)
                nc.scalar.copy(out=st0[:, 0, P:TH], in_=gps[:, 0, P:TH])
                stt = st0
            else:
                st1 = stp.tile([P, 4, TH], BF, tag="st1", name=f"st1_{l}")
                nc.scalar.copy(out=st1[:, 0:2, :], in_=gps[:, 0:2, :])
                nc.vector.tensor_mul(st1[:, 2, 0:P], gps[:, 2, 0:P], maskd)
                nc.vector.tensor_mul(st1[:, 3, P:TH], gps[:, 3, P:TH], maskd)
                nc.scalar.copy(out=st1[:, 2, P:TH], in_=gps[:, 2, P:TH])
                stt = st1

            # ---------- D: yKV rows of this half, then LN ------------------
            # one PSUM tile per jl chunk: Tile's PE-write vs engine-read
            # hazard is tile-granular, so with a single tile jl1's matmuls
            # would serialize behind the vector engine's LN reads of jl0
            dpsc = [
                mmp.tile([P, D], FP32, tag="mm", name=f"dps{l}_{h}_{jl}")
                for jl in range(2)
            ]
            xbf0 = ST["xbf"][0]
            xbf1 = ST["xbf"].get(1)
            yln = yp.tile([P, 2, D], BF, tag="yln", name=f"yln{l}_{h}")
            ylnT = yp.tile([P, DT, TH], BF, tag="ylnT", name=f"ylnT{l}_{h}")
            tp4 = mmp.tile([P, 2, DT, P], BF, tag="mm", name=f"ytp{l}_{h}")
            for jl in range(2):
                jp = h * 2 + jl
                for i in range(jp + 1):
                    if h == 0:
                        lhs = stt[:, i, ds(jp * P, P)]
                    else:
                        lhs = stt[:, i, ds(jl * P, P)]
                    rhs = xbf0[:, i, :] if i < 2 else xbf1[:, i - 2, :]
                    # each jl chunk owns its own tile/bank: i==0 sets start
                    nc.tensor.matmul(
                        dpsc[jl][:],
                        lhsT=lhs,
                        rhs=rhs,
                        start=(i == 0),
                        stop=(i == jp),
                    )
                # LN of chunk jl overlaps chunk jl+1's matmuls (vector side);
                # the apply runs on ScalarE so the exposed jl1 chain is short
                mv = ln_stats(dpsc[jl][:], f"d{l}_{h}_{jl}")
                r = rstd_of(mv[:, 1:2], f"d{l}_{h}_{jl}")
                nb = neg_mb(mv[:, 0:1], r, f"d{l}_{h}_{jl}")
                nc.scalar.activation(
                    out=yln[:, jl, :], in_=dpsc[jl][:], func=AF.Identity,
                    bias=nb, scale=r,
                )
            # transposes AFTER both D chunks so they never head-block D's
            # matmuls in the in-order tensor queue. jl-major, with E's first
            # group split by output-column half and interleaved: the jl0
            # columns of E group 0 run while yln jl1's LN chain is still in
            # flight, hiding the exposed D -> E serial handoff.
            evq0, _ = EVC[0]
            ps0 = aqp.tile([P, 4, TH], FP32, tag="aq", name=f"eps{l}_{h}_0")
            for jl in range(2):
                for dt_ in range(DT):
                    nc.tensor.transpose(
                        tp4[:, jl, dt_, :], yln[:, jl, ds(dt_ * P, P)], ident
                    )
                for dt_ in range(DT):
                    nc.scalar.copy(
                        out=ylnT[:, dt_, ds(jl * P, P)],
                        in_=tp4[:, jl, dt_, :],
                    )
                for j in range(G4):
                    for dt_ in range(DT):
                        # bank-first MMs (j 0/2, jl0, dt0) set start; the
                        # whole-bank has_written clear covers the siblings
                        nc.tensor.matmul(
                            ps0[:, j, ds(jl * P, P)],
                            lhsT=evq0[:, dt_, j, :],
                            rhs=ylnT[:, dt_, ds(jl * P, P)],
                            start=(dt_ == 0 and jl == 0 and j % 2 == 0),
                            stop=(dt_ == DT - 1),
                        )

            # ---------- E (gated y_sparse) + F (decoder), fused per group --
            fps = mmp.tile([P, 2, D], FP32, tag="mm", name=f"fps{l}_{h}")
            for g in range(NG):
                evq, decq = EVC[g // 4]
                if g == 0:
                    ps = ps0
                else:
                    ps = aqp.tile(
                        [P, 4, TH], FP32, tag="aq", name=f"eps{l}_{h}_{g}"
                    )
                    for j in range(G4):
                        for dt_ in range(DT):
                            nc.tensor.matmul(
                                ps[:, j, :],
                                lhsT=evq[:, dt_, (g % 4) * G4 + j, :],
                                rhs=ylnT[:, dt_, :],
                                start=(dt_ == 0),
                                stop=(dt_ == DT - 1),
                            )
                ys = yp.tile([P, G4, TH], BF, tag="ys", name=f"ys{l}_{h}_{g}")
                nc.scalar.activation(out=ys, in_=ps, func=AF.Relu)
                # trigger chunk c+2 once chunk c's readers are provably done,
                # so the WAR wait on the scalar HWDGE ring never stalls
                if g == 4:
                    EVC[2] = load_ev_chunk(l, h, 2)
                elif g == 8:
                    EVC[3] = load_ev_chunk(l, h, 3)
                xy = xyp.tile([P, G4, TH], BF, tag="xy", name=f"xy{l}_{h}_{g}")
                nc.vector.tensor_mul(
                    xy, ys, vall[:, A_IDX[g // 2], ds((g % 2) * G4, G4), :]
                )
                for i in range(G4):
                    k = g * G4 + i
                    for m in range(2):
                        # fps is one bank: only (k==0, m==0) may set start
                        nc.tensor.matmul(
                            fps[:, m, :],
                            lhsT=xy[:, i, ds(m * P, P)],
                            rhs=decq[:, k % 16, :],
                            start=(k == 0 and m == 0),
                            stop=(k == NT - 1),
                        )

            # ---------- AllReduce of this half's yMLP partial --------------
            ymlp = yp.tile([P, 2, D], BF, tag="ym", name=f"ym{l}_{h}")
            nc.scalar.copy(out=ymlp, in_=fps)
            cc_in = drm.tile([P, 2, D], BF, tag=f"ci{h}", name=f"ci{l}_{h}")
            cc_out = drm.tile([P, 2, D], BF, tag=f"co{h}", name=f"co{l}_{h}")
            nc.gpsimd.dma_start(out=cc_in[:], in_=ymlp)
            nc.gpsimd.collective_compute(
                "AllReduce", ALU.add, replica_groups=RG,
                ins=[cc_in[:]], outs=[cc_out[:]],
            )
            ymr = yp.tile([P, 2, D], BF, tag=f"ymr{h}", name=f"ymr{l}_{h}")
            nc.sync.dma_start(out=ymr, in_=cc_out[:])
            ST["ymr"][h] = ymr

        _QR0: dict = {}

        # startup: enc chunks first on the scalar ring (storage order == A's
        # consumption order, so A(0,0) starts once chunk 0 lands ~3us in);
        # x0 rides the sync ring in parallel, warmup AR on gpsimd
        for ch in range(8):
            nc.scalar.dma_start(
                out=enc_sb[:, :, ds(ch * 8, 8), :],
                in_=enc_d[:, :, ds(ch * 8, 8), :],
            )
        for h in range(2):
            emit_x_from_dram(h)
        zt = sb.tile([P, 2, D], BF, name="zt")
        nc.vector.memset(zt, 0.0)
        w_in = drm.tile([P, 2, D], BF, tag="wi", name="w_in")
        w_out = drm.tile([P, 2, D], BF, tag="wo", name="w_out")
        nc.gpsimd.dma_start(out=w_in[:], in_=zt)
        nc.gpsimd.collective_compute(
            "AllReduce", ALU.add, replica_groups=RG, ins=[w_in[:]], outs=[w_out[:]]
        )
        nc.gpsimd.dma_start(out=zt, in_=w_out[:])  # sink back into zt
        # PE warmup against DMA jitter at the very start; extra batches are
        # paced by the x0T arrivals so the HAM activity window stays busy
        # across the enc-load stretch instead of re-throttling to 4/8
        for w in range(8):
            wtp = aqp.tile([P, 4, TH], BF, tag="aq", name=f"wtp{w}")
            nc.tensor.transpose(wtp[:, 0, 0:P], ident, ident)
        for h in range(2):
            for w in range(8):
                wtp = aqp.tile([P, 4, TH], BF, tag="aq", name=f"wtx{h}_{w}")
                nc.tensor.transpose(wtp[:, 0, 0:P], ST["xT"][h][:, 0, 0:P], ident)

        for l in range(N_LAYER):
            for h in range(2):
                if l > 0:
                    emit_x_update(l, h)
                emit_half(l, h)

        # ---------- final x update + lm head ------------------------------
        for h in range(2):
            emit_x_update(N_LAYER, h)
            xT = ST["xT"][h]
            # one PSUM tile per jl so jl0's evacuation copy never serializes
            # jl1's matmuls; per-jl copy + output DMA shortens the
            # post-AllReduce serial tail
            lout = yp.tile([P, 2, VOCAB], FP32, tag="lout", name=f"lout{h}")
            for jl in range(2):
                lps = mmp.tile([P, VOCAB], FP32, tag="mm", name=f"lps{h}_{jl}")
                for dt_ in range(DT):
                    nc.tensor.matmul(
                        lps[:],
                        lhsT=xT[:, dt_, ds(jl * P, P)],
                        rhs=lmt[:, dt_, :],
                        start=(dt_ == 0),
                        stop=(dt_ == DT - 1),
                    )
                nc.scalar.copy(out=lout[:, jl, :], in_=lps[:])
                nc.sync.dma_start(
                    out=out_d[:, ds(h * 2 + jl, 1), :], in_=lout[:, ds(jl, 1), :]
                )

    if not nc.is_finalized():
        nc.finalize()
    return nc


def _ln_np(x):
    m = x.mean(-1, keepdims=True)
    v = ((x - m) ** 2).mean(-1, keepdims=True)
    return (x - m) / np.sqrt(v + EPS)


def _make_tables():
    t = np.arange(N, dtype=np.float32)
    q = np.floor(t / 2.0) * 2.0
    freqs = (1.0 / (THETA ** (q / N)) / (2.0 * np.float32(math.pi))).astype(
        np.float32
    )
    phases = np.arange(T, dtype=np.float32)[:, None] * freqs[None, :]
    ph = np.float32(np.float32(phases % 1.0) * np.float32(2.0 * math.pi))
    return np.cos(ph).astype(np.float32), np.sin(ph).astype(np.float32)


def _prep_inputs(idx, embed_w, encoder, encoder_v, decoder, lm_head):
    perm = np.concatenate([np.arange(HALF) * 2, np.arange(HALF) * 2 + 1])

    cos, sin = _make_tables()
    # both tables are column-periodic with period HALF after deinterleave
    cos_half = cos[:, perm[:HALF]]  # (T, HALF)
    sin_half = sin[:, perm[:HALF]]  # positive table; sign folded into ops
    cos_h = np.ascontiguousarray(
        cos_half.T.reshape(NT // 2, P, T).transpose(1, 0, 2)
    ).astype(BF16)  # [P, 32, T]
    sin_h = np.ascontiguousarray(
        sin_half.T.reshape(NT // 2, P, T).transpose(1, 0, 2)
    ).astype(BF16)
    # pack cos+sin per rope group so each (group, half) is ONE dma:
    # tab[:, gm, 0:4, :] = cos rows, tab[:, gm, 4:8, :] = sin rows
    tab_h = np.empty((P, 8, 2 * G4, T), BF16)
    for gm in range(8):
        tab_h[:, gm, 0:G4, :] = cos_h[:, gm * G4 : (gm + 1) * G4, :]
        tab_h[:, gm, G4:, :] = sin_h[:, gm * G4 : (gm + 1) * G4, :]

    # strict-upper 128x128 block mask: keep (p, c) when c > p
    mask_h = (np.arange(P)[None, :] > np.arange(P)[:, None]).astype(BF16)

    lm_h = np.ascontiguousarray(
        lm_head.reshape(DT, P, VOCAB).transpose(1, 0, 2)
    ).astype(BF16)

    x0 = _ln_np(embed_w[idx].astype(np.float32))  # (B, T, D)

    dec3 = decoder.reshape(NH, N, D)

    per_core = []
    for core in range(N_CORES):
        b, h = divmod(core, NH)
        enc_p = encoder[h][:, perm]  # (D, N)
        encv_p = encoder_v[h][:, perm]
        dec_p = dec3[h][perm, :]  # (N, D)

        enc_h = np.ascontiguousarray(
            enc_p.reshape(DT, P, NT, P).transpose(1, 0, 2, 3)
        ).astype(BF16)
        # reorder n-tiles so storage order matches A's A_VG consumption order
        a_order = np.concatenate(
            [np.arange(vg * VG, (vg + 1) * VG) for vg in A_VG]
        )
        enc_h = np.ascontiguousarray(enc_h[:, :, a_order, :])
        encv_h = np.ascontiguousarray(
            encv_p.reshape(DT, P, NT, P).transpose(1, 0, 2, 3)
        ).astype(BF16)
        dec_h = np.ascontiguousarray(
            dec_p.reshape(NT, P, D).transpose(1, 0, 2)
        ).astype(BF16)

        xb = x0[b]  # (T, D) f32
        x0_c = np.ascontiguousarray(
            xb.reshape(TC, P, D).transpose(1, 0, 2)
        ).astype(np.float32)
        x0bf_c = x0_c.astype(BF16)
        x0T_c = np.ascontiguousarray(
            xb.T.reshape(DT, P, T).transpose(1, 0, 2)
        ).astype(BF16)

        per_core.append(
            {
                "x0": x0_c,
                "x0bf": x0bf_c,
                "x0T": x0T_c,
                "enc": enc_h,
                "encv": encv_h,
                "dec": dec_h,
                "tabb": tab_h,
                "maskb": mask_h,
                "lm": lm_h,
            }
        )
    return per_core


def _get_nc():
    if "nc" not in _CACHE:
        _CACHE["nc"] = _build_bass()
    return _CACHE["nc"]


def kernel(idx, embed_w, encoder, encoder_v, decoder, lm_head, **extra):
    idx = np.asarray(idx)
    embed_w = np.asarray(embed_w, dtype=np.float32)
    encoder = np.asarray(encoder, dtype=np.float32)
    encoder_v = np.asarray(encoder_v, dtype=np.float32)
    decoder = np.asarray(decoder, dtype=np.float32)
    lm_head = np.asarray(lm_head, dtype=np.float32)

    nc = _get_nc()
    in_maps = _prep_inputs(idx, embed_w, encoder, encoder_v, decoder, lm_head)
    res = run_bass_kernel_spmd(nc, in_maps, core_ids=list(range(N_CORES)))
    _CACHE["last_results"] = res

    out = np.zeros((B, T, VOCAB), np.float32)
    for b in range(B):
        lg = res.results[b * NH]["logits"]  # [P, TC, VOCAB]
        out[b] = lg.transpose(1, 0, 2).reshape(T, VOCAB)
    return out


if __name__ == "__main__":
    rng = np.random.default_rng(0)
    ins = {
        "idx": rng.integers(0, VOCAB, (B, T)).astype(np.int32),
        "embed_w": (0.02 * rng.standard_normal((VOCAB, D))).astype(np.float32),
        "encoder": (0.02 * rng.standard_normal((NH, D, N))).astype(np.float32),
        "encoder_v": (0.02 * rng.standard_normal((NH, D, N))).astype(np.float32),
        "decoder": (0.02 * rng.standard_normal((NH * N, D))).astype(np.float32),
        "lm_head": (0.02 * rng.standard_normal((D, VOCAB))).astype(np.float32),
    }
    out = kernel(**ins)
    print("out", out.shape, out.dtype, float(np.abs(out).max()))



# revision 35
# speedup vs baseline: 1.0107x; 1.0107x over previous
"""Trainium2 Bass kernel for nn_BDH_39127152067244 (dense_transformer).

Sharding: 8 cores = (b, h) pairs — b = core // 4, h = core % 4. Each core
computes its head's share of every layer; the only cross-core communication
is a 4-rank AllReduce of the per-head yMLP partial (replica groups {0..3}
and {4..7}), issued once per t-half per layer.

Structure: each layer is emitted as two serialized t-half streams
(t in [0,256) then [256,512)). The AllReduce for half h is issued right
after that half's decoder matmul finishes and is consumed only at the START
of the next layer's half-h stream — a full opposite-half stream (~60us) is
always in flight between issue and use, so collective latency is hidden and
the PE never idles long enough to drop to the cold HAM clock. A dummy
AllReduce at kernel start absorbs the expensive first-collective path.

Layout tricks (vs the reference):
  - The N axis (8192) is deinterleaved on the host (even n first, odd n
    second). Rope's interleaved pair-swap becomes a half-offset of whole
    128-partition tiles. Both rope tables are then column-periodic with
    period N/2, so only [T, 4096] of cos and sin are stored/streamed.
  - x_sparse lives in ONE [P, 8, 8, TH] tile in A_VG order so each rope
    pair's two V-blocks are adjacent; rope runs as two merged [P,2,4,TH]
    muls against the natural/block-reversed table view plus two combines.
  - x_sparse is computed directly in transposed [N, T] layout.
  - Per-jl PSUM tiles for yKV and the lm head (Tile's PE-write vs
    engine-read hazard is tile-granular; separate tiles let jl1 matmuls
    overlap jl0's LN). LN rstd is one fused Abs_reciprocal_sqrt ACTIVATE;
    LN applies run on ScalarE with a precomputed -mean*rstd bias.
  - encv/dec chunks prefetch at half start; enc loads in 8 chunks ordered
    as A consumes them, with HAM-warming filler transposes during startup.
  - scores: the Gram matrix of rope'd activations is symmetric, so the
    strict-lower-triangular masked scores in [t, s] layout equal the
    strict-upper masked Gram in [s, t] layout — computed directly as the
    yKV matmul's lhsT. Only diagonal 128x128 blocks are masked; fully-kept
    blocks are copied and fully-masked blocks never computed.
  - All matmuls run in bf16 with f32 PSUM accumulation; LayerNorms and the
    residual stream stay f32.
"""

import math
import sys
from contextlib import ExitStack

import numpy as np
import ml_dtypes

sys.path.insert(0, "/opt/trn_rl_repo")

import concourse.bass as bass  # noqa: E402
import concourse.bacc as bacc  # noqa: E402
import concourse.mybir as mybir  # noqa: E402
import concourse.tile as tile  # noqa: E402
from concourse.bass import ds  # noqa: E402
from concourse.bass_utils import run_bass_kernel_spmd  # noqa: E402
from concourse.masks import make_identity  # noqa: E402

BF16 = ml_dtypes.bfloat16
BF = mybir.dt.bfloat16
FP32 = mybir.dt.float32
AF = mybir.ActivationFunctionType
ALU = mybir.AluOpType

# Problem constants (hardcoded per the harness contract).
N_LAYER = 6
D = 256
NH = 4
N = 8192
HALF = N // 2
VOCAB = 256
B, T = 2, 512
THETA = 2.0**16
EPS = 1e-5

P = 128          # partitions
NT = N // P      # 64 n-tiles
G4 = 4           # n-tiles per rope group
NG = NT // G4    # 16 groups
VG = 8           # n-tiles per V tile
NVG = NT // VG   # 8 V tiles
TC = T // P      # 4 t-chunks
TH = T // 2      # 256 columns per half
DT = D // P      # 2 d-tiles
N_CORES = 8
RG = [[0, 1, 2, 3], [4, 5, 6, 7]]

# rope pair emission order: (g, g^8) adjacent so the shared table tile is
# used twice back to back; A-phase V-tile order matches.
PAIR_G = [0, 8, 1, 9, 2, 10, 3, 11, 4, 12, 5, 13, 6, 14, 7, 15]
A_VG = [0, 4, 1, 5, 2, 6, 3, 7]
A_IDX = [A_VG.index(v) for v in range(8)]  # vg value -> V_all slot
GP_OFF = 0  # h0 rope pair-groups whose sin-mul runs on gpsimd (2.6us/op
# on gpsimd vs 0.59 on DVE made it the pacer — keep 0)

_CACHE: dict = {}


def _build_bass():
    nc = bacc.Bacc("TRN2", num_devices=N_CORES)

    x0_d = nc.dram_tensor("x0", [P, TC, D], FP32, kind="ExternalInput")
    x0bf_d = nc.dram_tensor("x0bf", [P, TC, D], BF, kind="ExternalInput")
    x0T_d = nc.dram_tensor("x0T", [P, DT, T], BF, kind="ExternalInput")
    enc_d = nc.dram_tensor("enc", [P, DT, NT, P], BF, kind="ExternalInput")
    encv_d = nc.dram_tensor("encv", [P, DT, NT, P], BF, kind="ExternalInput")
    dec_d = nc.dram_tensor("dec", [P, NT, D], BF, kind="ExternalInput")
    tab_d = nc.dram_tensor("tabb", [P, 8, 2 * G4, T], BF, kind="ExternalInput")
    mask_d = nc.dram_tensor("maskb", [P, P], BF, kind="ExternalInput")
    lm_d = nc.dram_tensor("lm", [P, DT, VOCAB], BF, kind="ExternalInput")
    out_d = nc.dram_tensor("logits", [P, TC, VOCAB], FP32, kind="ExternalOutput")

    with tile.TileContext(nc) as tc, ExitStack() as ctx:
        sb = ctx.enter_context(tc.tile_pool(name="sb", bufs=1))
        wres = ctx.enter_context(tc.tile_pool(name="wres", bufs=1))
        vpool = ctx.enter_context(tc.tile_pool(name="vpool", bufs=8))
        qr0p = ctx.enter_context(tc.tile_pool(name="qr0p", bufs=8))
        qr1p = ctx.enter_context(tc.tile_pool(name="qr1p", bufs=2))
        tmpp = ctx.enter_context(tc.tile_pool(name="tmpp", bufs=2))
        tabp = ctx.enter_context(tc.tile_pool(name="tabp", bufs=4))
        evp = ctx.enter_context(tc.tile_pool(name="evp", bufs=2))
        decp = ctx.enter_context(tc.tile_pool(name="decp", bufs=2))
        stp = ctx.enter_context(tc.tile_pool(name="stp", bufs=1))
        yp = ctx.enter_context(tc.tile_pool(name="yp", bufs=2))
        xyp = ctx.enter_context(tc.tile_pool(name="xyp", bufs=2))
        xp = ctx.enter_context(tc.tile_pool(name="xp", bufs=2))
        statp = ctx.enter_context(tc.tile_pool(name="statp", bufs=4))
        aqp = ctx.enter_context(tc.tile_pool(name="aqp", bufs=2, space="PSUM"))
        mmp = ctx.enter_context(tc.tile_pool(name="mmp", bufs=2, space="PSUM"))
        drm = ctx.enter_context(tc.tile_pool(name="drm", bufs=2, space="DRAM"))

        ident = sb.tile([P, P], BF, name="ident")
        make_identity(nc, ident)
        epst = sb.tile([P, 1], FP32, name="epst")
        nc.vector.memset(epst, EPS)
        maskd = sb.tile([P, P], BF, name="maskd")
        nc.sync.dma_start(out=maskd, in_=mask_d[:])
        lmt = sb.tile([P, DT, VOCAB], BF, name="lmt")
        nc.sync.dma_start(out=lmt, in_=lm_d[:])

        enc_sb = wres.tile([P, DT, NT, P], BF, name="enc_sb")

        # state carried between phases: per half h -> tile
        ST = {"ymr": {}, "xf": {}, "xbf": {}, "xT": {}}

        def rstd_of(var_ap, name):
            r = statp.tile([P, 1], FP32, tag="rs", name=f"rs_{name}")
            # fused 1/sqrt(|var+eps|): one scalar op, no DVE hop (Rsqrt is
            # gated in bass; the abs variant isn't, and var+eps > 0 anyway)
            nc.scalar.activation(
                out=r, in_=var_ap, func=AF.Abs_reciprocal_sqrt, bias=epst
            )
            return r

        def neg_mb(mean_ap, r_ap, name):
            """-mean*rstd, the ACTIVATE bias for a fused LN apply."""
            nb = statp.tile([P, 1], FP32, tag="nb", name=f"nb_{name}")
            nc.vector.scalar_tensor_tensor(
                out=nb, in0=mean_ap, scalar=-1.0, in1=r_ap,
                op0=ALU.mult, op1=ALU.mult,
            )
            return nb

        def ln_stats(src_ap, name):
            s6 = statp.tile([P, 6], FP32, tag="bst", name=f"st_{name}")
            nc.vector.bn_stats(out=s6, in_=src_ap)
            mv = statp.tile([P, 2], FP32, tag="bmv", name=f"mv_{name}")
            nc.vector.bn_aggr(out=mv, in_=s6)
            return mv

        def emit_x_from_dram(h):
            xf = xp.tile([P, 2, D], FP32, tag=f"xf{h}", name=f"xf0_{h}")
            nc.sync.dma_start(out=xf, in_=x0_d[:, ds(h * 2, 2), :])
            xbf = xp.tile([P, 2, D], BF, tag=f"xbf{h}", name=f"xbf0_{h}")
            nc.sync.dma_start(out=xbf, in_=x0bf_d[:, ds(h * 2, 2), :])
            xT = xp.tile([P, DT, TH], BF, tag=f"xT{h}", name=f"xT0_{h}")
            nc.sync.dma_start(out=xT, in_=x0T_d[:, :, ds(h * TH, TH)])
            ST["xf"][h], ST["xbf"][h], ST["xT"][h] = xf, xbf, xT

        def emit_x_update(l, h):
            """x_new = LN(x_old + LN(ymr)); consumes AR(l-1, h)."""
            ymr = ST["ymr"][h]
            xf_old = ST["xf"][h]
            xf = xp.tile([P, 2, D], FP32, tag=f"xf{h}", name=f"xf{l}_{h}")
            xbf = xp.tile([P, 2, D], BF, tag=f"xbf{h}", name=f"xbf{l}_{h}")
            xmid = xp.tile([P, 2, D], FP32, tag="xmid", name=f"xm{l}_{h}", bufs=1)
            xT = xp.tile([P, DT, TH], BF, tag=f"xT{h}", name=f"xT{l}_{h}")
            tp4x = mmp.tile([P, 2, DT, P], BF, tag="mm", name=f"xtp{l}_{h}")
            for jl in range(2):
                mv1 = ln_stats(ymr[:, jl, :], f"y{l}_{h}_{jl}")
                r1 = rstd_of(mv1[:, 1:2], f"y{l}_{h}_{jl}")
                # mean-shift of LN(ymr) is absorbed by the outer LN
                nc.vector.scalar_tensor_tensor(
                    out=xmid[:, jl, :],
                    in0=ymr[:, jl, :],
                    scalar=r1,
                    in1=xf_old[:, jl, :],
                    op0=ALU.mult,
                    op1=ALU.add,
                )
                mv2 = ln_stats(xmid[:, jl, :], f"x{l}_{h}_{jl}")
                r2 = rstd_of(mv2[:, 1:2], f"x{l}_{h}_{jl}")
                nb2 = neg_mb(mv2[:, 0:1], r2, f"x{l}_{h}_{jl}")
                # LN apply as two scalar ACTIVATEs (fp32 + bf16 outputs):
                # frees the DVE and lets xbf (the transpose input) materialize
                # without waiting for xf
                nc.scalar.activation(
                    out=xbf[:, jl, :], in_=xmid[:, jl, :], func=AF.Identity,
                    bias=nb2, scale=r2,
                )
                nc.vector.tensor_scalar(
                    out=xf[:, jl, :],
                    in0=xmid[:, jl, :],
                    scalar1=mv2[:, 0:1],
                    scalar2=r2,
                    op0=ALU.subtract,
                    op1=ALU.mult,
                )
                # per-jl transposes: jl0's run on PE while jl1's LN chain is
                # still on the vector engine
                for dt_ in range(DT):
                    nc.tensor.transpose(
                        tp4x[:, jl, dt_, :], xbf[:, jl, ds(dt_ * P, P)], ident
                    )
            for dt_ in range(DT):
                nc.scalar.copy(
                    out=xT[:, dt_, :].rearrange("p (a b) -> p a b", a=2),
                    in_=tp4x[:, :, dt_, :],
                )
            ST["xf"][h], ST["xbf"][h], ST["xT"][h] = xf, xbf, xT

        def load_ev_chunk(l, h, c):
            """Stream one 16-n-tile chunk of encv+dec for half (l, h)."""
            evq = evp.tile([P, DT, 16, P], BF, tag="ev", name=f"ev{l}_{h}_{c}")
            nc.scalar.dma_start(out=evq, in_=encv_d[:, :, ds(c * 16, 16), :])
            decq = decp.tile([P, 16, D], BF, tag="dec", name=f"dq{l}_{h}_{c}")
            nc.scalar.dma_start(out=decq, in_=dec_d[:, ds(c * 16, 16), :])
            return evq, decq

        def emit_half(l, h):
            xT = ST["xT"][h]

            # prefetch this half's first two encv/dec chunks (their pool
            # slots were freed by the previous half's last chunks), so the
            # D -> E handoff never waits on HBM. For (0,0) defer until after
            # A's emission: at startup the HBM port is saturated loading enc
            # and the rope tables, which are needed first.
            EVC = {}
            if not (l == 0 and h == 0):
                EVC[0] = load_ev_chunk(l, h, 0)
                EVC[1] = load_ev_chunk(l, h, 1)

            # ---------- A: V = relu(enc^T @ x^T) on this half's columns ----
            # one tile, A_VG-ordered along dim 1 so each rope pair's two
            # V-blocks are adjacent and a single [P,2,4,TH] AP spans them
            vall = vpool.tile(
                [P, 8, VG, TH], BF, tag="v", name=f"v{l}_{h}", bufs=1
            )
            for vgi, vg in enumerate(A_VG):
                if l == 0 and h == 0 and vgi >= 1:
                    # HAM-warming filler while enc chunks stream in
                    for w in range(6):
                        wt0 = aqp.tile(
                            [P, 4, TH], BF, tag="aq", name=f"aw{vgi}_{w}"
                        )
                        nc.tensor.transpose(wt0[:, 0, 0:P], ident, ident)
                for q in range(2):
                    ps = aqp.tile([P, 4, TH], FP32, tag="aq", name=f"aps{l}_{h}_{vg}_{q}")
                    for j in range(4):
                        # enc storage order is A_VG order (host-reordered)
                        nt_ = vgi * VG + q * 4 + j
                        for dt_ in range(DT):
                            nc.tensor.matmul(
                                ps[:, j, :],
                                lhsT=enc_sb[:, dt_, nt_, :],
                                rhs=xT[:, dt_, :],
                                start=(dt_ == 0),
                                stop=(dt_ == DT - 1),
                            )
                    if h == 1 and q == 1 and vgi % 2 == 1:
                        # shift a few h1 relu evacuations to the DVE (it has
                        # slack there) so ScalarE never stalls A's matmuls
                        nc.vector.tensor_scalar_max(
                            vall[:, vgi, ds(q * 4, 4), :], ps, 0.0
                        )
                    else:
                        nc.scalar.activation(
                            out=vall[:, vgi, ds(q * 4, 4), :], in_=ps,
                            func=AF.Relu,
                        )
            if l == 0 and h == 0:
                EVC[0] = load_ev_chunk(l, h, 0)
                EVC[1] = load_ev_chunk(l, h, 1)

            # ---------- rope + Gram (C), merged per pair -------------------
            # pair j = rope groups (j, j+8); both share the (column-periodic)
            # table gm=j. The rotation runs as two merged [P,2,4,TH] muls
            # against the natural and block-reversed table views, then two
            # [P,4,TH] combines:
            #   m_a = [Vg|Vp]*[c|s],  m_b = [Vg|Vp]*[s|c]
            #   qr_g = m_a0 - m_a1,   qr_p = m_b0 + m_b1
            if h == 0:
                gps = mmp.tile([P, 2, TH], FP32, tag="mm", name=f"gps{l}_0")
            else:
                gps = mmp.tile([P, 4, TH], FP32, tag="mm", name=f"gps{l}_1")
            QR = {}
            for pj in range(8):
                tabg = tabp.tile(
                    [P, 2 * G4, TH], BF, tag="tab", name=f"tb{l}_{h}_{pj}"
                )
                nc.sync.dma_start(
                    out=tabg, in_=tab_d[:, pj, :, ds(h * TH, TH)]
                )
                tab2 = tabg[:].rearrange("p (a b) t -> p a b t", a=2)
                vpair = vall[:, ds(2 * (pj // 2), 2), ds((pj % 2) * G4, G4), :]
                pool = qr0p if h == 0 else qr1p
                qrp = pool.tile(
                    [P, 2, G4, TH], BF, tag=f"q{h}", name=f"qr{l}_{h}_{pj}"
                )
                QR[pj] = qrp
                c2 = tmpp.tile([P, 2, G4, TH], BF, tag="c2", name=f"c2_{l}_{h}_{pj}", bufs=1)
                s2 = tmpp.tile([P, 2, G4, TH], BF, tag="s2", name=f"s2_{l}_{h}_{pj}", bufs=1)
                nc.vector.tensor_mul(c2, vpair, tab2)
                nc.vector.tensor_mul(s2, vpair, tab2[:, ::-1, :, :])
                nc.vector.tensor_sub(qrp[:, 0], c2[:, 0], c2[:, 1])
                nc.vector.tensor_add(qrp[:, 1], s2[:, 0], s2[:, 1])

                # Gram accumulation for this pair's 8 k-tiles
                # NOTE: start=True clears has_written bits for the WHOLE psum
                # bank, so only the first matmul touching each bank may set it;
                # sibling regions in the same bank start with start=False and
                # rely on that clear (their first write then overwrites).
                # gps h0 [P,2,TH] = 1 bank; gps h1 [P,4,TH] = 2 banks (j01/j23).
                for hop in range(2):
                    for i in range(G4):
                        first = pj == 0 and hop == 0 and i == 0
                        stp_ = pj == 7 and hop == 1 and i == G4 - 1
                        qr = qrp[:, hop]
                        if h == 0:
                            # rows j=0 (cols 0:256), j=1 (cols 128:256)
                            nc.tensor.matmul(
                                gps[:, 0, :],
                                lhsT=qr[:, i, 0:P],
                                rhs=qr[:, i, :],
                                start=first,
                                stop=stp_,
                            )
                            nc.tensor.matmul(
                                gps[:, 1, P:TH],
                                lhsT=qr[:, i, P:TH],
                                rhs=qr[:, i, P:TH],
                                start=False,
                                stop=stp_,
                            )
                        else:
                            qr0 = _QR0[pj][:, hop]
                            for j in range(4):
                                lhs = (
                                    qr0[:, i, ds((j % 2) * P, P)]
                                    if j < 2
                                    else qr[:, i, ds((j - 2) * P, P)]
                                )
                                st_ = first and (j % 2 == 0)
                                if j < 3:
                                    nc.tensor.matmul(
                                        gps[:, j, :],
                                        lhsT=lhs,
                                        rhs=qr[:, i, :],
                                        start=st_,
                                        stop=stp_,
                                    )
                                else:
                                    nc.tensor.matmul(
                                        gps[:, 3, P:TH],
                                        lhsT=lhs,
                                        rhs=qr[:, i, P:TH],
                                        start=st_,
                                        stop=stp_,
                                    )
            if h == 0:
                _QR0.clear()
                _QR0.update(QR)

            # ---------- masked score extraction ----------------------------
            if h == 0:
                st0 = stp.tile([P, 2, TH], BF, tag="st0", name=f"st0_{l}")
                nc.vector.tensor_mul(st0[:, 0, 0:P], gps[:, 0, 0:P], maskd)
                nc.vector.tensor_mul(st0[:, 1, P:TH], gps[:, 1, P:TH], maskd)
                nc.scalar.copy(out=st0[:, 0, P:TH], in_=gps[:, 0, P:TH])
                stt = st0
            else:
                st1 = stp.tile([P, 4, TH], BF, tag="st1", name=f"st1_{l}")
                nc.scalar.copy(out=st1[:, 0:2, :], in_=gps[:, 0:2, :])
                nc.vector.tensor_mul(st1[:, 2, 0:P], gps[:, 2, 0:P], maskd)
                nc.vector.tensor_mul(st1[:, 3, P:TH], gps[:, 3, P:TH], maskd)
                nc.scalar.copy(out=st1[:, 2, P:TH], in_=gps[:, 2, P:TH])
                stt = st1

            # ---------- D: yKV rows of this half, then LN ------------------
            # one PSUM tile per jl chunk: Tile's PE-write vs engine-read
            # hazard is tile-granular, so with a single tile jl1's matmuls
            # would serialize behind the vector engine's LN reads of jl0
            dpsc = [
                mmp.tile([P, D], FP32, tag="mm", name=f"dps{l}_{h}_{jl}")
                for jl in range(2)
            ]
            xbf0 = ST["xbf"][0]
            xbf1 = ST["xbf"].get(1)
            yln = yp.tile([P, 2, D], BF, tag="yln", name=f"yln{l}_{h}")
            ylnT = yp.tile([P, DT, TH], BF, tag="ylnT", name=f"ylnT{l}_{h}")
            tp4 = mmp.tile([P, 2, DT, P], BF, tag="mm", name=f"ytp{l}_{h}")
            for jl in range(2):
                jp = h * 2 + jl
                for i in range(jp + 1):
                    if h == 0:
                        lhs = stt[:, i, ds(jp * P, P)]
                    else:
                        lhs = stt[:, i, ds(jl * P, P)]
                    rhs = xbf0[:, i, :] if i < 2 else xbf1[:, i - 2, :]
                    # each jl chunk owns its own tile/bank: i==0 sets start
                    nc.tensor.matmul(
                        dpsc[jl][:],
                        lhsT=lhs,
                        rhs=rhs,
                        start=(i == 0),
                        stop=(i == jp),
                    )
                # LN of chunk jl overlaps chunk jl+1's matmuls (vector side);
                # the apply runs on ScalarE so the exposed jl1 chain is short
                mv = ln_stats(dpsc[jl][:], f"d{l}_{h}_{jl}")
                r = rstd_of(mv[:, 1:2], f"d{l}_{h}_{jl}")
                nb = neg_mb(mv[:, 0:1], r, f"d{l}_{h}_{jl}")
                nc.scalar.activation(
                    out=yln[:, jl, :], in_=dpsc[jl][:], func=AF.Identity,
                    bias=nb, scale=r,
                )
            # transposes AFTER both D chunks so they never head-block D's
            # matmuls in the in-order tensor queue. jl-major, with E's first
            # group split by output-column half and interleaved: the jl0
            # columns of E group 0 run while yln jl1's LN chain is still in
            # flight, hiding the exposed D -> E serial handoff.
            evq0, _ = EVC[0]
            ps0 = aqp.tile([P, 4, TH], FP32, tag="aq", name=f"eps{l}_{h}_0")
            for jl in range(2):
                for dt_ in range(DT):
                    nc.tensor.transpose(
                        tp4[:, jl, dt_, :], yln[:, jl, ds(dt_ * P, P)], ident
                    )
                for dt_ in range(DT):
                    # vector-engine copy: ScalarE is the congested engine at
                    # this boundary (yln ACT + rstd + relus queue there)
                    nc.vector.tensor_copy(
                        ylnT[:, dt_, ds(jl * P, P)], tp4[:, jl, dt_, :]
                    )
                for j in range(G4):
                    for dt_ in range(DT):
                        # bank-first MMs (j 0/2, jl0, dt0) set start; the
                        # whole-bank has_written clear covers the siblings
                        nc.tensor.matmul(
                            ps0[:, j, ds(jl * P, P)],
                            lhsT=evq0[:, dt_, j, :],
                            rhs=ylnT[:, dt_, ds(jl * P, P)],
                            start=(dt_ == 0 and jl == 0 and j % 2 == 0),
                            stop=(dt_ == DT - 1),
                        )

            # ---------- E (gated y_sparse) + F (decoder), fused per group --
            fps = mmp.tile([P, 2, D], FP32, tag="mm", name=f"fps{l}_{h}")
            for g in range(NG):
                evq, decq = EVC[g // 4]
                if g == 0:
                    ps = ps0
                else:
                    ps = aqp.tile(
                        [P, 4, TH], FP32, tag="aq", name=f"eps{l}_{h}_{g}"
                    )
                    for j in range(G4):
                        for dt_ in range(DT):
                            nc.tensor.matmul(
                                ps[:, j, :],
                                lhsT=evq[:, dt_, (g % 4) * G4 + j, :],
                                rhs=ylnT[:, dt_, :],
                                start=(dt_ == 0),
                                stop=(dt_ == DT - 1),
                            )
                ys = yp.tile([P, G4, TH], BF, tag="ys", name=f"ys{l}_{h}_{g}")
                nc.scalar.activation(out=ys, in_=ps, func=AF.Relu)
                # trigger chunk c+2 once chunk c's readers are provably done,
                # so the WAR wait on the scalar HWDGE ring never stalls
                if g == 4:
                    EVC[2] = load_ev_chunk(l, h, 2)
                elif g == 8:
                    EVC[3] = load_ev_chunk(l, h, 3)
                xy = xyp.tile([P, G4, TH], BF, tag="xy", name=f"xy{l}_{h}_{g}")
                nc.vector.tensor_mul(
                    xy, ys, vall[:, A_IDX[g // 2], ds((g % 2) * G4, G4), :]
                )
                for i in range(G4):
                    k = g * G4 + i
                    for m in range(2):
                        # fps is one bank: only (k==0, m==0) may set start
                        nc.tensor.matmul(
                            fps[:, m, :],
                            lhsT=xy[:, i, ds(m * P, P)],
                            rhs=decq[:, k % 16, :],
                            start=(k == 0 and m == 0),
                            stop=(k == NT - 1),
                        )

            # ---------- AllReduce of this half's yMLP partial --------------
            ymlp = yp.tile([P, 2, D], BF, tag="ym", name=f"ym{l}_{h}")
            nc.scalar.copy(out=ymlp, in_=fps)
            cc_in = drm.tile([P, 2, D], BF, tag=f"ci{h}", name=f"ci{l}_{h}")
            cc_out = drm.tile([P, 2, D], BF, tag=f"co{h}", name=f"co{l}_{h}")
            nc.gpsimd.dma_start(out=cc_in[:], in_=ymlp)
            nc.gpsimd.collective_compute(
                "AllReduce", ALU.add, replica_groups=RG,
                ins=[cc_in[:]], outs=[cc_out[:]],
            )
            ymr = yp.tile([P, 2, D], BF, tag=f"ymr{h}", name=f"ymr{l}_{h}")
            nc.sync.dma_start(out=ymr, in_=cc_out[:])
            ST["ymr"][h] = ymr

        _QR0: dict = {}

        # startup: enc chunks first on the scalar ring (storage order == A's
        # consumption order, so A(0,0) starts once chunk 0 lands ~3us in);
        # x0 rides the sync ring in parallel, warmup AR on gpsimd
        for ch in range(8):
            nc.scalar.dma_start(
                out=enc_sb[:, :, ds(ch * 8, 8), :],
                in_=enc_d[:, :, ds(ch * 8, 8), :],
            )
        for h in range(2):
            emit_x_from_dram(h)
        zt = sb.tile([P, 2, D], BF, name="zt")
        nc.vector.memset(zt, 0.0)
        w_in = drm.tile([P, 2, D], BF, tag="wi", name="w_in")
        w_out = drm.tile([P, 2, D], BF, tag="wo", name="w_out")
        nc.gpsimd.dma_start(out=w_in[:], in_=zt)
        nc.gpsimd.collective_compute(
            "AllReduce", ALU.add, replica_groups=RG, ins=[w_in[:]], outs=[w_out[:]]
        )
        nc.gpsimd.dma_start(out=zt, in_=w_out[:])  # sink back into zt
        # PE warmup against DMA jitter at the very start; extra batches are
        # paced by the x0T arrivals so the HAM activity window stays busy
        # across the enc-load stretch instead of re-throttling to 4/8
        for w in range(8):
            wtp = aqp.tile([P, 4, TH], BF, tag="aq", name=f"wtp{w}")
            nc.tensor.transpose(wtp[:, 0, 0:P], ident, ident)
        for h in range(2):
            for w in range(8):
                wtp = aqp.tile([P, 4, TH], BF, tag="aq", name=f"wtx{h}_{w}")
                nc.tensor.transpose(wtp[:, 0, 0:P], ST["xT"][h][:, 0, 0:P], ident)

        for l in range(N_LAYER):
            for h in range(2):
                if l > 0:
                    emit_x_update(l, h)
                emit_half(l, h)

        # ---------- final x update + lm head ------------------------------
        for h in range(2):
            emit_x_update(N_LAYER, h)
            xT = ST["xT"][h]
            # one PSUM tile per jl so jl0's evacuation copy never serializes
            # jl1's matmuls; per-jl copy + output DMA shortens the
            # post-AllReduce serial tail
            lout = yp.tile([P, 2, VOCAB], FP32, tag="lout", name=f"lout{h}")
            for jl in range(2):
                lps = mmp.tile([P, VOCAB], FP32, tag="mm", name=f"lps{h}_{jl}")
                for dt_ in range(DT):
                    nc.tensor.matmul(
                        lps[:],
                        lhsT=xT[:, dt_, ds(jl * P, P)],
                        rhs=lmt[:, dt_, :],
                        start=(dt_ == 0),
                        stop=(dt_ == DT - 1),
                    )
                nc.scalar.copy(out=lout[:, jl, :], in_=lps[:])
                nc.sync.dma_start(
                    out=out_d[:, ds(h * 2 + jl, 1), :], in_=lout[:, ds(jl, 1), :]
                )

    if not nc.is_finalized():
        nc.finalize()
    return nc


def _ln_np(x):
    m = x.mean(-1, keepdims=True)
    v = ((x - m) ** 2).mean(-1, keepdims=True)
    return (x - m) / np.sqrt(v + EPS)


def _make_tables():
    t = np.arange(N, dtype=np.float32)
    q = np.floor(t / 2.0) * 2.0
    freqs = (1.0 / (THETA ** (q / N)) / (2.0 * np.float32(math.pi))).astype(
        np.float32
    )
    phases = np.arange(T, dtype=np.float32)[:, None] * freqs[None, :]
    ph = np.float32(np.float32(phases % 1.0) * np.float32(2.0 * math.pi))
    return np.cos(ph).astype(np.float32), np.sin(ph).astype(np.float32)


def _prep_inputs(idx, embed_w, encoder, encoder_v, decoder, lm_head):
    perm = np.concatenate([np.arange(HALF) * 2, np.arange(HALF) * 2 + 1])

    cos, sin = _make_tables()
    # both tables are column-periodic with period HALF after deinterleave
    cos_half = cos[:, perm[:HALF]]  # (T, HALF)
    sin_half = sin[:, perm[:HALF]]  # positive table; sign folded into ops
    cos_h = np.ascontiguousarray(
        cos_half.T.reshape(NT // 2, P, T).transpose(1, 0, 2)
    ).astype(BF16)  # [P, 32, T]
    sin_h = np.ascontiguousarray(
        sin_half.T.reshape(NT // 2, P, T).transpose(1, 0, 2)
    ).astype(BF16)
    # pack cos+sin per rope group so each (group, half) is ONE dma:
    # tab[:, gm, 0:4, :] = cos rows, tab[:, gm, 4:8, :] = sin rows
    tab_h = np.empty((P, 8, 2 * G4, T), BF16)
    for gm in range(8):
        tab_h[:, gm, 0:G4, :] = cos_h[:, gm * G4 : (gm + 1) * G4, :]
        tab_h[:, gm, G4:, :] = sin_h[:, gm * G4 : (gm + 1) * G4, :]

    # strict-upper 128x128 block mask: keep (p, c) when c > p
    mask_h = (np.arange(P)[None, :] > np.arange(P)[:, None]).astype(BF16)

    lm_h = np.ascontiguousarray(
        lm_head.reshape(DT, P, VOCAB).transpose(1, 0, 2)
    ).astype(BF16)

    x0 = _ln_np(embed_w[idx].astype(np.float32))  # (B, T, D)

    dec3 = decoder.reshape(NH, N, D)

    per_core = []
    for core in range(N_CORES):
        b, h = divmod(core, NH)
        enc_p = encoder[h][:, perm]  # (D, N)
        encv_p = encoder_v[h][:, perm]
        dec_p = dec3[h][perm, :]  # (N, D)

        enc_h = np.ascontiguousarray(
            enc_p.reshape(DT, P, NT, P).transpose(1, 0, 2, 3)
        ).astype(BF16)
        # reorder n-tiles so storage order matches A's A_VG consumption order
        a_order = np.concatenate(
            [np.arange(vg * VG, (vg + 1) * VG) for vg in A_VG]
        )
        enc_h = np.ascontiguousarray(enc_h[:, :, a_order, :])
        encv_h = np.ascontiguousarray(
            encv_p.reshape(DT, P, NT, P).transpose(1, 0, 2, 3)
        ).astype(BF16)
        dec_h = np.ascontiguousarray(
            dec_p.reshape(NT, P, D).transpose(1, 0, 2)
        ).astype(BF16)

        xb = x0[b]  # (T, D) f32
        x0_c = np.ascontiguousarray(
            xb.reshape(TC, P, D).transpose(1, 0, 2)
        ).astype(np.float32)
        x0bf_c = x0_c.astype(BF16)
        x0T_c = np.ascontiguousarray(
            xb.T.reshape(DT, P, T).transpose(1, 0, 2)
        ).astype(BF16)

        per_core.append(
            {
                "x0": x0_c,
                "x0bf": x0bf_c,
                "x0T": x0T_c,
                "enc": enc_h,
                "encv": encv_h,
                "dec": dec_h,
                "tabb": tab_h,
                "maskb": mask_h,
                "lm": lm_h,
            }
        )
    return per_core


def _get_nc():
    if "nc" not in _CACHE:
        _CACHE["nc"] = _build_bass()
    return _CACHE["nc"]


def kernel(idx, embed_w, encoder, encoder_v, decoder, lm_head, **extra):
    idx = np.asarray(idx)
    embed_w = np.asarray(embed_w, dtype=np.float32)
    encoder = np.asarray(encoder, dtype=np.float32)
    encoder_v = np.asarray(encoder_v, dtype=np.float32)
    decoder = np.asarray(decoder, dtype=np.float32)
    lm_head = np.asarray(lm_head, dtype=np.float32)

    nc = _get_nc()
    in_maps = _prep_inputs(idx, embed_w, encoder, encoder_v, decoder, lm_head)
    res = run_bass_kernel_spmd(nc, in_maps, core_ids=list(range(N_CORES)))
    _CACHE["last_results"] = res

    out = np.zeros((B, T, VOCAB), np.float32)
    for b in range(B):
        lg = res.results[b * NH]["logits"]  # [P, TC, VOCAB]
        out[b] = lg.transpose(1, 0, 2).reshape(T, VOCAB)
    return out


if __name__ == "__main__":
    rng = np.random.default_rng(0)
    ins = {
        "idx": rng.integers(0, VOCAB, (B, T)).astype(np.int32),
        "embed_w": (0.02 * rng.standard_normal((VOCAB, D))).astype(np.float32),
        "encoder": (0.02 * rng.standard_normal((NH, D, N))).astype(np.float32),
        "encoder_v": (0.02 * rng.standard_normal((NH, D, N))).astype(np.float32),
        "decoder": (0.02 * rng.standard_normal((NH * N, D))).astype(np.float32),
        "lm_head": (0.02 * rng.standard_normal((D, VOCAB))).astype(np.float32),
    }
    out = kernel(**ins)
    print("out", out.shape, out.dtype, float(np.abs(out).max()))

